# revision 13
# baseline (speedup 1.0000x reference)
"""Multi-head attention (B=4, T=2048, D=1024, H=16) on 8 TRN2 NeuronCores.

Sharding: core c handles batch b = c//2 and head-half hh = c%2 (8 heads,
512 of the 1024 channel dims). Each core computes its half of the head
outputs and a row-sharded output projection, producing a partial
[T, D] output. Host unshard: out[b] = partial[2b] + partial[2b+1]
+ b_o + b_v @ w_o.T (the value-bias contribution commutes through
attention because softmax rows sum to 1).

v8: flat software-pipelined emission, ACT-saturating schedule.
  - The ACT exp stream (256 x [128,1024] activations, ~1.3us each) is the
    critical engine; the driver emits one attention step per exp and
    weaves all projection work into PE slack between steps.
  - Startup critical path: only xk/xq of t-block 0 plus the jb0 weight
    slices (~2.5MB) gate the first exp. Host pre-tiles inputs so every
    DMA source is a contiguous block.
  - tq0's attention is chunked by key-t-block (SBUF accumulation) so it
    streams while K/V of t-blocks 1-3 are still being projected.
  - Scores pairs run concurrently in PE row groups 0/64; softmax
    denominator rides row 64 of the AV accumulators (ones column in V);
    a K=1 PE matmul broadcasts it across partitions.
"""

from contextlib import ExitStack

import numpy as np
import ml_dtypes

import concourse.bass as bass
import concourse.mybir as mybir
import concourse.tile as tile
from concourse import bacc
from concourse.bass_utils import run_bass_kernel_spmd

B, T, D = 4, 2048, 1024
H = 16
DH = 64  # head dim
HALF = 512  # channels per core (8 heads)
N_CORES = 8

F32 = mybir.dt.float32
BF16 = mybir.dt.bfloat16

TB = 512  # t-block for moving operands
NTB = T // TB  # 4
KB = 128  # contraction block
NKB = D // KB  # 8
NJB = HALF // KB  # 4 j-blocks of the half
NTK = T // KB  # 16 tk blocks


class Step:
    __slots__ = ("tq", "jp", "tk", "seg_first", "seg_last", "final", "sc", "ex", "seg")

    def __init__(self, tq, jp, tk, seg_first, seg_last, final):
        self.tq = tq
        self.jp = jp
        self.tk = tk
        self.seg_first = seg_first  # first step of an av accumulation segment
        self.seg_last = seg_last  # last step of an av accumulation segment
        self.final = final  # last segment of this (tq, jp): normalize after
        self.sc = None
        self.ex = None
        self.seg = None


def build_kernel():
    nc = bacc.Bacc(
        "TRN2", target_bir_lowering=False, debug=False, num_devices=N_CORES
    )
    # pre-tiled inputs: x*[kb][tb] -> [128, 512] contiguous blocks
    xq = nc.dram_tensor("xq", [NTB * KB, NKB * TB], BF16, kind="ExternalInput").ap()
    xk = nc.dram_tensor("xk", [NTB * KB, NKB * TB], BF16, kind="ExternalInput").ap()
    xv = nc.dram_tensor("xv", [NTB * KB, NKB * TB], BF16, kind="ExternalInput").ap()
    # wq/wk tiled [jb][kb] -> [128, 128] contiguous blocks
    wq = nc.dram_tensor("wq", [NJB * KB, NKB * KB], BF16, kind="ExternalInput").ap()
    wk = nc.dram_tensor("wk", [NJB * KB, NKB * KB], BF16, kind="ExternalInput").ap()
    # wv rows contiguous per kb block; wo rows contiguous per jb block
    wv = nc.dram_tensor("wv", [KB, NKB * HALF], BF16, kind="ExternalInput").ap()
    wo = nc.dram_tensor("wo", [KB, NJB * D], BF16, kind="ExternalInput").ap()
    bq = nc.dram_tensor("bq", [HALF, 1], F32, kind="ExternalInput").ap()
    bk = nc.dram_tensor("bk", [HALF, 1], F32, kind="ExternalInput").ap()
    ones_in = nc.dram_tensor("ones_in", [KB, H // 2], BF16, kind="ExternalInput").ap()
    ones_bc_in = nc.dram_tensor(
        "ones_bc_in", [DH + 1, DH], BF16, kind="ExternalInput"
    ).ap()
    partial = nc.dram_tensor("partial", [T, D], F32, kind="ExternalOutput").ap()

    with tile.TileContext(nc) as tc, ExitStack() as ctx:
        p_const = ctx.enter_context(tc.tile_pool(name="const", bufs=1))
        p_kt = ctx.enter_context(tc.tile_pool(name="kt", bufs=NJB * NTB))
        p_v = ctx.enter_context(tc.tile_pool(name="v", bufs=NTK))
        p_qt = ctx.enter_context(tc.tile_pool(name="qt", bufs=2 * NJB))
        p_xs = ctx.enter_context(tc.tile_pool(name="xs", bufs=5))
        p_ex = ctx.enter_context(tc.tile_pool(name="ex", bufs=8))
        p_ot = ctx.enter_context(tc.tile_pool(name="ot", bufs=2 * NJB))
        p_as = ctx.enter_context(tc.tile_pool(name="as", bufs=6))
        p_ac = ctx.enter_context(tc.tile_pool(name="ac", bufs=8))
        p_rc = ctx.enter_context(tc.tile_pool(name="rc", bufs=3))
        p_st = ctx.enter_context(tc.tile_pool(name="st", bufs=2))
        # PSUM: scores 2x[128,1024] (4 banks) + av 2x[65,512] + pj 2x[128,512]
        p_sc = ctx.enter_context(tc.tile_pool(name="sc", bufs=2, space="PSUM"))
        p_av = ctx.enter_context(tc.tile_pool(name="av", bufs=2, space="PSUM"))
        p_pj = ctx.enter_context(tc.tile_pool(name="pj", bufs=2, space="PSUM"))

        # ---- persistent SBUF tiles ----
        w_k = p_const.tile([KB, NKB, HALF], BF16, tag="wk")
        w_q = p_const.tile([KB, NKB, HALF], BF16, tag="wq")
        w_v = p_const.tile([KB, NKB, HALF], BF16, tag="wv")
        w_o = p_const.tile([KB, NJB, D], BF16, tag="wo")
        b_k = p_const.tile([KB, NJB], F32, tag="bk")
        b_q = p_const.tile([KB, NJB], F32, tag="bq")
        ones8 = p_const.tile([KB, H // 2], BF16, tag="ones8")
        ones_bc = p_const.tile([DH + 1, DH], BF16, tag="onesbc")
        warm = p_const.tile([1, 8], F32, tag="warm")
        warm_o = p_const.tile([1, 8], BF16, tag="warmo")

        kt_tiles = [
            [p_kt.tile([KB, TB], BF16, tag="kt", name=f"kt{j}_{tb}") for tb in range(NTB)]
            for j in range(NJB)
        ]
        v_tiles = [
            p_v.tile([KB, H // 2, DH + 1], BF16, tag="v", name=f"v{j}")
            for j in range(NTK)
        ]

        # ---- DMA emitters ----
        def dma_w_jb(dst, src, jb):
            # one jb slice of wq/wk in a single DMA: [128, kb 8, 128]
            nc.sync.dma_start(
                dst[:, :, jb * KB : (jb + 1) * KB],
                src[jb * KB : (jb + 1) * KB, :].rearrange(
                    "p (kb j) -> p kb j", j=KB
                ),
            )

        def load_x_tile(src, tb):
            xt = p_xs.tile([KB, NKB, TB], BF16, tag="xs")
            nc.sync.dma_start(
                xt[:],
                src[tb * KB : (tb + 1) * KB, :].rearrange(
                    "p (kb t) -> p kb t", t=TB
                ),
            )
            return xt

        # ---- projection chain emitters (each returns two half-chain thunks
        # so the driver can spread chains across pipeline steps) ----
        xk_tiles = {}  # tb -> tiles (shared by the 4 jb chains)
        xq_tiles = {}
        xv_tiles = {}
        HC = NKB // 2  # MMs per half-chain

        def k_parts(tb, jb):
            st = {}

            def a():
                if tb not in xk_tiles:
                    xk_tiles[tb] = load_x_tile(xk, tb)
                st["ps"] = p_pj.tile([KB, TB], F32, tag="pj", name="ps")
                for kb in range(HC):
                    nc.tensor.matmul(
                        st["ps"][:],
                        w_k[:, kb, jb * KB : (jb + 1) * KB],
                        xk_tiles[tb][:, kb, :],
                        start=(kb == 0),
                        stop=False,
                    )

            def b():
                for kb in range(HC, NKB):
                    nc.tensor.matmul(
                        st["ps"][:],
                        w_k[:, kb, jb * KB : (jb + 1) * KB],
                        xk_tiles[tb][:, kb, :],
                        start=False,
                        stop=(kb == NKB - 1),
                    )
                nc.vector.tensor_scalar_add(
                    kt_tiles[jb][tb][:], st["ps"][:], b_k[:, jb : jb + 1]
                )
                if jb == NJB - 1:
                    del xk_tiles[tb]

            return a, b

        def v_parts(tb, ts):
            st = {}

            def a():
                if tb not in xv_tiles:
                    for u in range(4):
                        nc.vector.tensor_copy(
                            v_tiles[tb * 4 + u][:, :, DH : DH + 1],
                            ones8[:, :, None],
                        )
                    xv_tiles[tb] = load_x_tile(xv, tb)
                st["ps"] = p_pj.tile([KB, TB], F32, tag="pj", name="ps")
                for kb in range(HC):
                    nc.tensor.matmul(
                        st["ps"][:],
                        xv_tiles[tb][:, kb, ts * KB : (ts + 1) * KB],
                        w_v[:, kb, :],
                        start=(kb == 0),
                        stop=False,
                    )

            def b():
                for kb in range(HC, NKB):
                    nc.tensor.matmul(
                        st["ps"][:],
                        xv_tiles[tb][:, kb, ts * KB : (ts + 1) * KB],
                        w_v[:, kb, :],
                        start=False,
                        stop=(kb == NKB - 1),
                    )
                nc.vector.tensor_copy(
                    v_tiles[tb * 4 + ts][:, :, 0:DH],
                    st["ps"][:].rearrange("p (h d) -> p h d", d=DH),
                )
                if ts == 3:
                    del xv_tiles[tb]

            return a, b

        qt_gen = {}  # tq -> list of qt tiles

        def q_parts(tq, jb):
            st = {}

            def a():
                if tq not in qt_gen:
                    qt_gen[tq] = [None] * NJB
                    xq_tiles[tq] = load_x_tile(xq, tq)
                st["ps"] = p_pj.tile([KB, TB], F32, tag="pj", name="ps")
                for kb in range(HC):
                    nc.tensor.matmul(
                        st["ps"][:],
                        w_q[:, kb, jb * KB : (jb + 1) * KB],
                        xq_tiles[tq][:, kb, :],
                        start=(kb == 0),
                        stop=False,
                    )

            def b():
                for kb in range(HC, NKB):
                    nc.tensor.matmul(
                        st["ps"][:],
                        w_q[:, kb, jb * KB : (jb + 1) * KB],
                        xq_tiles[tq][:, kb, :],
                        start=False,
                        stop=(kb == NKB - 1),
                    )
                qt = p_qt.tile([KB, TB], BF16, tag="qt", name=f"qt{jb}")
                nc.vector.tensor_scalar_add(qt[:], st["ps"][:], b_q[:, jb : jb + 1])
                qt_gen[tq][jb] = qt
                if jb == NJB - 1:
                    del xq_tiles[tq]

            return a, b

        # ---- attention step emitters ----
        def emit_sc(d):
            d.sc = p_sc.tile([KB, 2 * TB], F32, tag="sc", name="sc")
            qt = qt_gen[d.tq]
            for i in range(2):
                nc.tensor.matmul(
                    d.sc[:, i * TB : (i + 1) * TB],
                    kt_tiles[d.jp][d.tk // 4][
                        i * DH : (i + 1) * DH,
                        (d.tk % 4) * KB : (d.tk % 4 + 1) * KB,
                    ],
                    qt[d.jp][i * DH : (i + 1) * DH, :],
                    start=True,
                    stop=True,
                )

        def emit_exp(d):
            d.ex = p_ex.tile([KB, 2 * TB], BF16, tag="ex", name="ex")
            nc.scalar.activation(
                d.ex[:], d.sc[:], mybir.ActivationFunctionType.Exp, scale=0.125
            )

        seg_avs = {}  # (tq, jp) -> current av psum pair

        def emit_av(d):
            if d.seg_first:
                seg_avs[d.tq, d.jp] = [
                    p_av.tile([DH + 1, TB], F32, tag="av", name=f"av{i}")
                    for i in range(2)
                ]
            avs = seg_avs[d.tq, d.jp]
            for i in range(2):
                nc.tensor.matmul(
                    avs[i][:],
                    v_tiles[d.tk][:, 2 * d.jp + i, :],
                    d.ex[:, i * TB : (i + 1) * TB],
                    start=d.seg_first,
                    stop=d.seg_last,
                )

        acc = {}  # (tq, jp, i) -> sbuf accumulator (chunked tq only)
        ot_gen = {}  # tq -> ot tiles

        def emit_spill(d, first_chunk):
            """Chunked path: move/add av psum pair into SBUF accumulators."""
            avs = seg_avs.pop((d.tq, d.jp))
            for i in range(2):
                if first_chunk:
                    acc[d.tq, d.jp, i] = p_ac.tile(
                        [DH + 1, TB], F32, tag="ac", name=f"ac{d.jp}_{i}"
                    )
                    nc.vector.tensor_copy(acc[d.tq, d.jp, i][:], avs[i][:])
                else:
                    nc.vector.tensor_add(
                        acc[d.tq, d.jp, i][:], acc[d.tq, d.jp, i][:], avs[i][:]
                    )

        def normalize(tq, jp, i, src):
            """src: [65, TB] f32 head-pair output; row 64 = denominator."""
            ot_tiles = ot_gen[tq]
            db = p_as.tile([DH + 1, TB], BF16, tag="db")
            nc.vector.tensor_copy(db[DH : DH + 1, :], src[DH : DH + 1, :])
            dbc = p_av.tile([DH, TB], F32, tag="av", name="dbc")
            nc.tensor.matmul(
                dbc[:],
                ones_bc[DH : DH + 1, :],
                db[DH : DH + 1, :],
                start=True,
                stop=True,
            )
            rc2 = p_rc.tile([DH, TB], F32, tag="rc2")
            nc.vector.reciprocal_approx_fast(rc2[:], dbc[:])
            if i == 0:
                nc.vector.tensor_mul(ot_tiles[jp][0:DH, :], src[0:DH, :], rc2[:])
            else:
                stg = p_rc.tile([DH, TB], BF16, tag="stg")
                nc.vector.tensor_mul(stg[:], src[0:DH, :], rc2[:])
                nc.sync.dma_start(ot_tiles[jp][DH : 2 * DH, :], stg[:])

        def emit_norm(d):
            """Final segment of (tq, jp): drain av psum / acc and normalize."""
            if d.tq == 0:
                emit_spill(d, False)
                srcs = [acc[d.tq, d.jp, i] for i in range(2)]
            else:
                avs = seg_avs.pop((d.tq, d.jp))
                srcs = []
                for i in range(2):
                    a = p_as.tile([DH + 1, TB], F32, tag="as")
                    nc.vector.tensor_copy(a[:], avs[i][:])
                    srcs.append(a)
            for i in (1, 0):
                normalize(d.tq, d.jp, i, srcs[i])

        def out_chain(tq, nb, ts):
            def emit():
                ot_tiles = ot_gen[tq]
                po = p_pj.tile([KB, TB], F32, tag="pj")
                for jp in range(NJB):
                    nc.tensor.matmul(
                        po[:],
                        ot_tiles[jp][:, ts * KB : (ts + 1) * KB],
                        w_o[:, jp, nb * TB : (nb + 1) * TB],
                        start=(jp == 0),
                        stop=(jp == NJB - 1),
                    )
                st = p_st.tile([KB, TB], F32, tag="st")
                nc.vector.tensor_copy(st[:], po[:])
                nc.sync.dma_start(
                    partial[
                        tq * TB + ts * KB : tq * TB + (ts + 1) * KB,
                        nb * TB : (nb + 1) * TB,
                    ],
                    st[:],
                )

            return emit

        # ---- build the global step list ----
        steps = []
        for tq in range(NTB):
            if tq == 0:
                for ci in range(4):
                    for jp in range(NJB):
                        for u in range(4):
                            steps.append(
                                Step(tq, jp, ci * 4 + u, u == 0, u == 3, ci == 3)
                            )
            else:
                for jp in range(NJB):
                    for u in range(NTK):
                        steps.append(Step(tq, jp, u, u == 0, u == NTK - 1, True))

        # ---- per-step filler plan: fillers[s] emitted between exp(s) and
        # sc(s+1).  Deadlines: a chain feeding sc(s+1) must be in fillers[<=s];
        # one feeding av(s-LAG) must be in fillers[<=s] as well (av comes last).
        AV_LAG = 6
        fillers = [[] for _ in steps]

        def place(s, thunk):
            fillers[min(max(s, 0), len(steps) - 1)].append(thunk)

        # tq0: K/Q chains jb1-3 (sc(4*jb) lookahead at step 4*jb-1)
        for j, s0 in [(1, 0), (2, 4), (3, 8)]:
            ka, kb_ = k_parts(0, j)
            qa, qb = q_parts(0, j)
            place(s0, ka)
            place(s0 + 1, kb_)
            place(s0 + 2, qa)
            place(s0 + 3, qb)
        # V tb0: av(u) runs at step u+AV_LAG
        for u in range(4):
            va, vb = v_parts(0, u)
            place(2 + 2 * u, va)
            place(3 + 2 * u, vb)
        # K/V of tb 1-3: kt[jp][tb] first read by sc at step 16*tb+4*jp-1;
        # v_tiles[4*tb+u] first read by av at step 16*tb+u+AV_LAG.
        for tb in range(1, NTB):
            for j in range(NJB):
                ka, kb_ = k_parts(tb, j)
                place(16 * tb - 6 + 2 * j, ka)
                place(16 * tb - 5 + 2 * j, kb_)
            for u in range(4):
                va, vb = v_parts(tb, u)
                place(16 * tb + 2 + 2 * u, va)
                place(16 * tb + 3 + 2 * u, vb)
        place(30, lambda: nc.sync.dma_start(
            w_o[:], wo.rearrange("p (jb n) -> p jb n", n=D)))
        # q_proj(tq+1) near the end of tq; out_proj(tq-1) spread across tq.
        for tq in range(NTB):
            base = tq * 64
            if tq + 1 < NTB:
                for jb in range(NJB):
                    qa, qb = q_parts(tq + 1, jb)
                    place(base + 56 + 2 * jb, qa)
                    place(base + 57 + 2 * jb, qb)
            if tq > 0:
                chains = [out_chain(tq - 1, nb, ts) for nb in range(2) for ts in range(4)]
                for idx, ch in enumerate(chains):
                    place(base + 6 + 4 * idx, ch)

        # ---- emission ----
        # ACT table pre-warm: a tiny exp long before the first real one
        nc.sync.dma_start(warm[:, 0:1], bq[0:1, 0:1])
        nc.scalar.activation(
            warm_o[:, 0:1], warm[:, 0:1], mybir.ActivationFunctionType.Exp, scale=1.0
        )

        # startup DMA critical path
        dma_w_jb(w_k, wk, 0)
        nc.sync.dma_start(b_k[:], bk.rearrange("(jb p) one -> p (jb one)", p=KB))
        ka0, kb0 = k_parts(0, 0)
        ka0()
        kb0()
        dma_w_jb(w_q, wq, 0)
        nc.sync.dma_start(b_q[:], bq.rearrange("(jb p) one -> p (jb one)", p=KB))
        qa0, qb0 = q_parts(0, 0)
        qa0()
        qb0()
        # next DMAs: remaining wk/wq jb slices, then V-path constants
        for jb in range(1, NJB):
            dma_w_jb(w_k, wk, jb)
            dma_w_jb(w_q, wq, jb)
        nc.sync.dma_start(ones8[:], ones_in[:])
        nc.sync.dma_start(ones_bc[:], ones_bc_in[:])
        nc.sync.dma_start(w_v[:], wv.rearrange("p (kb n) -> p kb n", n=HALF))

        # ---- the pipeline ----
        def av_and_drain(d):
            emit_av(d)
            if d.seg_last:
                if d.final:
                    emit_norm(d)
                elif d.tq == 0:
                    emit_spill(d, d.tk < 4)

        n_steps = len(steps)
        av_next = 0
        for s, d in enumerate(steps):
            if d.tq not in ot_gen:
                ot_gen[d.tq] = [
                    p_ot.tile([KB, TB], BF16, tag="ot", name=f"ot{j}")
                    for j in range(NJB)
                ]
            if s == 0:
                emit_sc(d)
            emit_exp(d)
            for thunk in fillers[s]:
                thunk()
            if s + 1 < n_steps:
                emit_sc(steps[s + 1])
            # av stream: lag AV_LAG normally, taper near the end so the
            # post-exp flush (which runs with ACT idle) is short
            want = s - AV_LAG + 1
            if s >= n_steps - 2 * AV_LAG:
                want = s - 1
            while av_next < min(max(want, 0), n_steps) and av_next <= s:
                av_and_drain(steps[av_next])
                av_next += 1
        while av_next < n_steps:
            av_and_drain(steps[av_next])
            av_next += 1
        # tail: out projection of the last t-block, two chains pipelined
        # so DVE copies/DMAs overlap the next pair's matmuls
        def out_pair(tq, specs):
            pos = [p_pj.tile([KB, TB], F32, tag="pj", name="po") for _ in specs]
            for jp in range(NJB):
                for ci, (nb, ts) in enumerate(specs):
                    nc.tensor.matmul(
                        pos[ci][:],
                        ot_gen[tq][jp][:, ts * KB : (ts + 1) * KB],
                        w_o[:, jp, nb * TB : (nb + 1) * TB],
                        start=(jp == 0),
                        stop=(jp == NJB - 1),
                    )
            for ci, (nb, ts) in enumerate(specs):
                st_t = p_st.tile([KB, TB], F32, tag="st", name="st_t")
                nc.vector.tensor_copy(st_t[:], pos[ci][:])
                nc.sync.dma_start(
                    partial[
                        tq * TB + ts * KB : tq * TB + (ts + 1) * KB,
                        nb * TB : (nb + 1) * TB,
                    ],
                    st_t[:],
                )

        tail_specs = [(nb, ts) for nb in range(2) for ts in range(4)]
        for k in range(0, 8, 2):
            out_pair(NTB - 1, tail_specs[k : k + 2])

    nc.compile()
    return nc


def kernel(**inputs: np.ndarray) -> np.ndarray:
    query = np.asarray(inputs["query"], dtype=np.float32)
    key = np.asarray(inputs["key"], dtype=np.float32)
    value = np.asarray(inputs["value"], dtype=np.float32)
    w_q = np.asarray(inputs["w_q"], dtype=np.float32)
    b_q = np.asarray(inputs["b_q"], dtype=np.float32)
    w_k = np.asarray(inputs["w_k"], dtype=np.float32)
    b_k = np.asarray(inputs["b_k"], dtype=np.float32)
    w_v = np.asarray(inputs["w_v"], dtype=np.float32)
    b_v = np.asarray(inputs["b_v"], dtype=np.float32)
    w_o = np.asarray(inputs["w_o"], dtype=np.float32)
    b_o = np.asarray(inputs["b_o"], dtype=np.float32)

    nc = build_kernel()

    bf = ml_dtypes.bfloat16

    def tile_x(a):
        # [T, D] -> transpose -> [kb][tb][128][512] contiguous
        at = a.T.astype(bf)  # [D, T]
        return np.ascontiguousarray(
            at.reshape(NKB, KB, NTB, TB).transpose(2, 1, 0, 3)
        ).reshape(NTB * KB, NKB * TB)

    def tile_w(w_sl):
        # w[sl,:] -> [D, HALF] transposed -> [jb][kb][128][128] contiguous
        wt = w_sl.T.astype(bf)  # [D, HALF]
        return np.ascontiguousarray(
            wt.reshape(NKB, KB, NJB, KB).transpose(2, 1, 0, 3)
        ).reshape(NJB * KB, NKB * KB)

    in_maps = []
    for c in range(N_CORES):
        b = c // 2
        hh = c % 2
        sl = slice(hh * HALF, (hh + 1) * HALF)
        in_maps.append(
            {
                "xq": tile_x(query[b]),
                "xk": tile_x(key[b]),
                "xv": tile_x(value[b]),
                "wq": tile_w(w_q[sl, :]),
                "wk": tile_w(w_k[sl, :]),
                "wv": np.ascontiguousarray(
                    w_v[sl, :].T.astype(bf).reshape(NKB, KB, HALF).transpose(1, 0, 2)
                ).reshape(KB, NKB * HALF),
                "wo": np.ascontiguousarray(
                    w_o[:, sl].T.astype(bf).reshape(NJB, KB, D).transpose(1, 0, 2)
                ).reshape(KB, NJB * D),
                "bq": np.ascontiguousarray(b_q[sl].reshape(HALF, 1)),
                "bk": np.ascontiguousarray(b_k[sl].reshape(HALF, 1)),
                "ones_in": np.ones((KB, H // 2), dtype=bf),
                "ones_bc_in": np.ones((DH + 1, DH), dtype=bf),
            }
        )

    res = run_bass_kernel_spmd(nc, in_maps, core_ids=list(range(N_CORES)))

    const_row = (b_v[None, :] @ w_o.T + b_o[None, :]).astype(np.float32)
    out = np.empty((B, T, D), dtype=np.float32)
    for b in range(B):
        out[b] = res.results[2 * b]["partial"] + res.results[2 * b + 1]["partial"]
        out[b] += const_row
    return out


# revision 14
# speedup vs baseline: 1.0019x; 1.0019x over previous
"""Multi-head attention (B=4, T=2048, D=1024, H=16) on 8 TRN2 NeuronCores.

Sharding: core c handles batch b = c//2 and head-half hh = c%2 (8 heads,
512 of the 1024 channel dims). Each core computes its half of the head
outputs and a row-sharded output projection, producing a partial
[T, D] output. Host unshard: out[b] = partial[2b] + partial[2b+1]
+ b_o + b_v @ w_o.T (the value-bias contribution commutes through
attention because softmax rows sum to 1).

v8: flat software-pipelined emission, ACT-saturating schedule.
  - The ACT exp stream (256 x [128,1024] activations, ~1.3us each) is the
    critical engine; the driver emits one attention step per exp and
    weaves all projection work into PE slack between steps.
  - Startup critical path: only xk/xq of t-block 0 plus the jb0 weight
    slices (~2.5MB) gate the first exp. Host pre-tiles inputs so every
    DMA source is a contiguous block.
  - tq0's attention is chunked by key-t-block (SBUF accumulation) so it
    streams while K/V of t-blocks 1-3 are still being projected.
  - Scores pairs run concurrently in PE row groups 0/64; softmax
    denominator rides row 64 of the AV accumulators (ones column in V);
    a K=1 PE matmul broadcasts it across partitions.
"""

from contextlib import ExitStack

import numpy as np
import ml_dtypes

import concourse.bass as bass
import concourse.mybir as mybir
import concourse.tile as tile
from concourse import bacc
from concourse.bass_utils import run_bass_kernel_spmd

B, T, D = 4, 2048, 1024
H = 16
DH = 64  # head dim
HALF = 512  # channels per core (8 heads)
N_CORES = 8

F32 = mybir.dt.float32
BF16 = mybir.dt.bfloat16

TB = 512  # t-block for moving operands
NTB = T // TB  # 4
KB = 128  # contraction block
NKB = D // KB  # 8
NJB = HALF // KB  # 4 j-blocks of the half
NTK = T // KB  # 16 tk blocks


class Step:
    __slots__ = ("tq", "jp", "tk", "seg_first", "seg_last", "final", "sc", "ex", "seg")

    def __init__(self, tq, jp, tk, seg_first, seg_last, final):
        self.tq = tq
        self.jp = jp
        self.tk = tk
        self.seg_first = seg_first  # first step of an av accumulation segment
        self.seg_last = seg_last  # last step of an av accumulation segment
        self.final = final  # last segment of this (tq, jp): normalize after
        self.sc = None
        self.ex = None
        self.seg = None


def build_kernel():
    nc = bacc.Bacc(
        "TRN2", target_bir_lowering=False, debug=False, num_devices=N_CORES
    )
    # pre-tiled inputs: x*[kb][tb] -> [128, 512] contiguous blocks
    xq = nc.dram_tensor("xq", [NTB * KB, NKB * TB], BF16, kind="ExternalInput").ap()
    xk = nc.dram_tensor("xk", [NTB * KB, NKB * TB], BF16, kind="ExternalInput").ap()
    xv = nc.dram_tensor("xv", [NTB * KB, NKB * TB], BF16, kind="ExternalInput").ap()
    # wq/wk tiled [jb][kb] -> [128, 128] contiguous blocks
    wq = nc.dram_tensor("wq", [NJB * KB, NKB * KB], BF16, kind="ExternalInput").ap()
    wk = nc.dram_tensor("wk", [NJB * KB, NKB * KB], BF16, kind="ExternalInput").ap()
    # wv rows contiguous per kb block; wo rows contiguous per jb block
    wv = nc.dram_tensor("wv", [KB, NKB * HALF], BF16, kind="ExternalInput").ap()
    wo = nc.dram_tensor("wo", [KB, NJB * D], BF16, kind="ExternalInput").ap()
    bq = nc.dram_tensor("bq", [HALF, 1], F32, kind="ExternalInput").ap()
    bk = nc.dram_tensor("bk", [HALF, 1], F32, kind="ExternalInput").ap()
    ones_in = nc.dram_tensor("ones_in", [KB, H // 2], BF16, kind="ExternalInput").ap()
    ones_bc_in = nc.dram_tensor(
        "ones_bc_in", [DH + 1, DH], BF16, kind="ExternalInput"
    ).ap()
    partial = nc.dram_tensor("partial", [T, D], F32, kind="ExternalOutput").ap()

    with tile.TileContext(nc) as tc, ExitStack() as ctx:
        p_const = ctx.enter_context(tc.tile_pool(name="const", bufs=1))
        p_kt = ctx.enter_context(tc.tile_pool(name="kt", bufs=NJB * NTB))
        p_v = ctx.enter_context(tc.tile_pool(name="v", bufs=NTK))
        p_qt = ctx.enter_context(tc.tile_pool(name="qt", bufs=2 * NJB))
        p_xs = ctx.enter_context(tc.tile_pool(name="xs", bufs=5))
        p_ex = ctx.enter_context(tc.tile_pool(name="ex", bufs=8))
        p_ot = ctx.enter_context(tc.tile_pool(name="ot", bufs=2 * NJB))
        p_as = ctx.enter_context(tc.tile_pool(name="as", bufs=6))
        p_ac = ctx.enter_context(tc.tile_pool(name="ac", bufs=8))
        p_rc = ctx.enter_context(tc.tile_pool(name="rc", bufs=3))
        p_st = ctx.enter_context(tc.tile_pool(name="st", bufs=2))
        # PSUM: scores 2x[128,1024] (4 banks) + av 2x[65,512] + pj 2x[128,512]
        p_sc = ctx.enter_context(tc.tile_pool(name="sc", bufs=2, space="PSUM"))
        p_av = ctx.enter_context(tc.tile_pool(name="av", bufs=2, space="PSUM"))
        p_pj = ctx.enter_context(tc.tile_pool(name="pj", bufs=2, space="PSUM"))

        # ---- persistent SBUF tiles ----
        w_k = p_const.tile([KB, NJB, NKB, KB], BF16, tag="wk")
        w_q = p_const.tile([KB, NJB, NKB, KB], BF16, tag="wq")
        w_v = p_const.tile([KB, NKB, HALF], BF16, tag="wv")
        w_o = p_const.tile([KB, NJB, D], BF16, tag="wo")
        b_k = p_const.tile([KB, NJB], F32, tag="bk")
        b_q = p_const.tile([KB, NJB], F32, tag="bq")
        ones8 = p_const.tile([KB, H // 2], BF16, tag="ones8")
        ones_bc = p_const.tile([DH + 1, DH], BF16, tag="onesbc")
        warm = p_const.tile([1, 8], F32, tag="warm")
        warm_o = p_const.tile([1, 8], BF16, tag="warmo")

        kt_tiles = [
            [p_kt.tile([KB, TB], BF16, tag="kt", name=f"kt{j}_{tb}") for tb in range(NTB)]
            for j in range(NJB)
        ]
        v_tiles = [
            p_v.tile([KB, H // 2, DH + 1], BF16, tag="v", name=f"v{j}")
            for j in range(NTK)
        ]

        # ---- DMA emitters ----
        def dma_w_jb(dst, src, jb):
            # one jb slice of wq/wk in a single DMA: [128, kb 8, 128]
            nc.sync.dma_start(
                dst[:, jb],
                src[jb * KB : (jb + 1) * KB, :].rearrange(
                    "p (kb j) -> p kb j", j=KB
                ),
            )

        def load_x_tile(src, tb):
            xt = p_xs.tile([KB, NKB, TB], BF16, tag="xs")
            nc.sync.dma_start(
                xt[:],
                src[tb * KB : (tb + 1) * KB, :].rearrange(
                    "p (kb t) -> p kb t", t=TB
                ),
            )
            return xt

        # ---- projection chain emitters (each returns two half-chain thunks
        # so the driver can spread chains across pipeline steps) ----
        xk_tiles = {}  # tb -> tiles (shared by the 4 jb chains)
        xq_tiles = {}
        xv_tiles = {}
        HC = NKB // 2  # MMs per half-chain

        def k_parts(tb, jb):
            st = {}

            def a():
                if tb not in xk_tiles:
                    xk_tiles[tb] = load_x_tile(xk, tb)
                st["ps"] = p_pj.tile([KB, TB], F32, tag="pj", name="ps")
                for kb in range(HC):
                    nc.tensor.matmul(
                        st["ps"][:],
                        w_k[:, jb, kb, :],
                        xk_tiles[tb][:, kb, :],
                        start=(kb == 0),
                        stop=False,
                    )

            def b():
                for kb in range(HC, NKB):
                    nc.tensor.matmul(
                        st["ps"][:],
                        w_k[:, jb, kb, :],
                        xk_tiles[tb][:, kb, :],
                        start=False,
                        stop=(kb == NKB - 1),
                    )
                nc.vector.tensor_scalar_add(
                    kt_tiles[jb][tb][:], st["ps"][:], b_k[:, jb : jb + 1]
                )
                if jb == NJB - 1:
                    del xk_tiles[tb]

            return a, b

        def v_parts(tb, ts):
            st = {}

            def a():
                if tb not in xv_tiles:
                    for u in range(4):
                        nc.vector.tensor_copy(
                            v_tiles[tb * 4 + u][:, :, DH : DH + 1],
                            ones8[:, :, None],
                        )
                    xv_tiles[tb] = load_x_tile(xv, tb)
                st["ps"] = p_pj.tile([KB, TB], F32, tag="pj", name="ps")
                for kb in range(HC):
                    nc.tensor.matmul(
                        st["ps"][:],
                        xv_tiles[tb][:, kb, ts * KB : (ts + 1) * KB],
                        w_v[:, kb, :],
                        start=(kb == 0),
                        stop=False,
                    )

            def b():
                for kb in range(HC, NKB):
                    nc.tensor.matmul(
                        st["ps"][:],
                        xv_tiles[tb][:, kb, ts * KB : (ts + 1) * KB],
                        w_v[:, kb, :],
                        start=False,
                        stop=(kb == NKB - 1),
                    )
                nc.vector.tensor_copy(
                    v_tiles[tb * 4 + ts][:, :, 0:DH],
                    st["ps"][:].rearrange("p (h d) -> p h d", d=DH),
                )
                if ts == 3:
                    del xv_tiles[tb]

            return a, b

        qt_gen = {}  # tq -> list of qt tiles

        def q_parts(tq, jb):
            st = {}

            def a():
                if tq not in qt_gen:
                    qt_gen[tq] = [None] * NJB
                if tq not in xq_tiles:
                    xq_tiles[tq] = load_x_tile(xq, tq)
                st["ps"] = p_pj.tile([KB, TB], F32, tag="pj", name="ps")
                for kb in range(HC):
                    nc.tensor.matmul(
                        st["ps"][:],
                        w_q[:, jb, kb, :],
                        xq_tiles[tq][:, kb, :],
                        start=(kb == 0),
                        stop=False,
                    )

            def b():
                for kb in range(HC, NKB):
                    nc.tensor.matmul(
                        st["ps"][:],
                        w_q[:, jb, kb, :],
                        xq_tiles[tq][:, kb, :],
                        start=False,
                        stop=(kb == NKB - 1),
                    )
                qt = p_qt.tile([KB, TB], BF16, tag="qt", name=f"qt{jb}")
                nc.vector.tensor_scalar_add(qt[:], st["ps"][:], b_q[:, jb : jb + 1])
                qt_gen[tq][jb] = qt
                if jb == NJB - 1:
                    del xq_tiles[tq]

            return a, b

        # ---- attention step emitters ----
        def emit_sc(d):
            d.sc = p_sc.tile([KB, 2 * TB], F32, tag="sc", name="sc")
            qt = qt_gen[d.tq]
            for i in range(2):
                nc.tensor.matmul(
                    d.sc[:, i * TB : (i + 1) * TB],
                    kt_tiles[d.jp][d.tk // 4][
                        i * DH : (i + 1) * DH,
                        (d.tk % 4) * KB : (d.tk % 4 + 1) * KB,
                    ],
                    qt[d.jp][i * DH : (i + 1) * DH, :],
                    start=True,
                    stop=True,
                )

        def emit_exp(d):
            d.ex = p_ex.tile([KB, 2 * TB], BF16, tag="ex", name="ex")
            nc.scalar.activation(
                d.ex[:], d.sc[:], mybir.ActivationFunctionType.Exp, scale=0.125
            )

        seg_avs = {}  # (tq, jp) -> current av psum pair

        def emit_av(d):
            if d.seg_first:
                seg_avs[d.tq, d.jp] = [
                    p_av.tile([DH + 1, TB], F32, tag="av", name=f"av{i}")
                    for i in range(2)
                ]
            avs = seg_avs[d.tq, d.jp]
            for i in range(2):
                nc.tensor.matmul(
                    avs[i][:],
                    v_tiles[d.tk][:, 2 * d.jp + i, :],
                    d.ex[:, i * TB : (i + 1) * TB],
                    start=d.seg_first,
                    stop=d.seg_last,
                )

        acc = {}  # (tq, jp, i) -> sbuf accumulator (chunked tq only)
        ot_gen = {}  # tq -> ot tiles

        def emit_spill(d, first_chunk):
            """Chunked path: move/add av psum pair into SBUF accumulators."""
            avs = seg_avs.pop((d.tq, d.jp))
            for i in range(2):
                if first_chunk:
                    acc[d.tq, d.jp, i] = p_ac.tile(
                        [DH + 1, TB], F32, tag="ac", name=f"ac{d.jp}_{i}"
                    )
                    nc.vector.tensor_copy(acc[d.tq, d.jp, i][:], avs[i][:])
                else:
                    nc.vector.tensor_add(
                        acc[d.tq, d.jp, i][:], acc[d.tq, d.jp, i][:], avs[i][:]
                    )

        def normalize(tq, jp, i, src):
            """src: [65, TB] f32 head-pair output; row 64 = denominator."""
            ot_tiles = ot_gen[tq]
            db = p_as.tile([DH + 1, TB], BF16, tag="db")
            nc.vector.tensor_copy(db[DH : DH + 1, :], src[DH : DH + 1, :])
            dbc = p_av.tile([DH, TB], F32, tag="av", name="dbc")
            nc.tensor.matmul(
                dbc[:],
                ones_bc[DH : DH + 1, :],
                db[DH : DH + 1, :],
                start=True,
                stop=True,
            )
            rc2 = p_rc.tile([DH, TB], F32, tag="rc2")
            nc.vector.reciprocal_approx_fast(rc2[:], dbc[:])
            if i == 0:
                nc.vector.tensor_mul(ot_tiles[jp][0:DH, :], src[0:DH, :], rc2[:])
            else:
                stg = p_rc.tile([DH, TB], BF16, tag="stg")
                nc.vector.tensor_mul(stg[:], src[0:DH, :], rc2[:])
                nc.sync.dma_start(ot_tiles[jp][DH : 2 * DH, :], stg[:])

        def emit_norm(d):
            """Final segment of (tq, jp): drain av psum / acc and normalize."""
            if d.tq == 0:
                emit_spill(d, False)
                srcs = [acc[d.tq, d.jp, i] for i in range(2)]
            else:
                avs = seg_avs.pop((d.tq, d.jp))
                srcs = []
                for i in range(2):
                    a = p_as.tile([DH + 1, TB], F32, tag="as")
                    nc.vector.tensor_copy(a[:], avs[i][:])
                    srcs.append(a)
            for i in (1, 0):
                normalize(d.tq, d.jp, i, srcs[i])

        def out_parts(tq, nb, ts):
            stt = {}

            def mm_half(lo, hi):
                for jp in range(lo, hi):
                    nc.tensor.matmul(
                        stt["po"][:],
                        ot_gen[tq][jp][:, ts * KB : (ts + 1) * KB],
                        w_o[:, jp, nb * TB : (nb + 1) * TB],
                        start=(jp == 0),
                        stop=(jp == NJB - 1),
                    )

            def a():
                stt["po"] = p_pj.tile([KB, TB], F32, tag="pj", name="po")
                mm_half(0, 2)

            def b():
                mm_half(2, NJB)
                st = p_st.tile([KB, TB], F32, tag="st")
                nc.vector.tensor_copy(st[:], stt["po"][:])
                nc.sync.dma_start(
                    partial[
                        tq * TB + ts * KB : tq * TB + (ts + 1) * KB,
                        nb * TB : (nb + 1) * TB,
                    ],
                    st[:],
                )

            return a, b

        # ---- build the global step list ----
        steps = []
        for tq in range(NTB):
            if tq == 0:
                for ci in range(4):
                    for jp in range(NJB):
                        for u in range(4):
                            steps.append(
                                Step(tq, jp, ci * 4 + u, u == 0, u == 3, ci == 3)
                            )
            else:
                for jp in range(NJB):
                    for u in range(NTK):
                        steps.append(Step(tq, jp, u, u == 0, u == NTK - 1, True))

        # ---- per-step filler plan: fillers[s] emitted between exp(s) and
        # sc(s+1).  Deadlines: a chain feeding sc(s+1) must be in fillers[<=s];
        # one feeding av(s-LAG) must be in fillers[<=s] as well (av comes last).
        AV_LAG = 6
        fillers = [[] for _ in steps]

        def place(s, thunk):
            fillers[min(max(s, 0), len(steps) - 1)].append(thunk)

        # tq0: K/Q chains jb1-3 (sc(4*jb) lookahead at step 4*jb-1)
        for j, s0 in [(1, 0), (2, 4), (3, 8)]:
            ka, kb_ = k_parts(0, j)
            qa, qb = q_parts(0, j)
            place(s0, ka)
            place(s0 + 1, kb_)
            place(s0 + 2, qa)
            place(s0 + 3, qb)
        # V tb0: av(u) runs at step u+AV_LAG
        for u in range(4):
            va, vb = v_parts(0, u)
            place(2 + 2 * u, va)
            place(3 + 2 * u, vb)
        def prefetch(xt_dict, dram, tb):
            def emit():
                if tb not in xt_dict:
                    xt_dict[tb] = load_x_tile(dram, tb)
            return emit

        # K/V of tb 1-3: kt[jp][tb] first read by sc at step 16*tb+4*jp-1;
        # v_tiles[4*tb+u] first read by av at step 16*tb+u+AV_LAG.
        for tb in range(1, NTB):
            place(16 * tb - 12, prefetch(xk_tiles, xk, tb))
            place(16 * tb - 4, prefetch(xv_tiles, xv, tb))
            for j in range(NJB):
                ka, kb_ = k_parts(tb, j)
                place(16 * tb - 6 + 2 * j, ka)
                place(16 * tb - 5 + 2 * j, kb_)
            for u in range(4):
                va, vb = v_parts(tb, u)
                place(16 * tb + 2 + 2 * u, va)
                place(16 * tb + 3 + 2 * u, vb)
        place(30, lambda: nc.sync.dma_start(
            w_o[:], wo.rearrange("p (jb n) -> p jb n", n=D)))
        # q_proj(tq+1) near the end of tq; out_proj(tq-1) spread across tq.
        for tq in range(NTB):
            base = tq * 64
            if tq + 1 < NTB:
                place(base + 44, prefetch(xq_tiles, xq, tq + 1))
                for jb in range(NJB):
                    qa, qb = q_parts(tq + 1, jb)
                    place(base + 50 + 3 * jb, qa)
                    place(base + 51 + 3 * jb, qb)
            if tq > 0:
                for idx, (nb, ts) in enumerate(
                    (nb, ts) for nb in range(2) for ts in range(4)
                ):
                    oa, ob = out_parts(tq - 1, nb, ts)
                    place(base + 5 + 4 * idx, oa)
                    place(base + 6 + 4 * idx, ob)

        # ---- emission ----
        # ACT table pre-warm: a tiny exp long before the first real one
        nc.sync.dma_start(warm[:, 0:1], bq[0:1, 0:1])
        nc.scalar.activation(
            warm_o[:, 0:1], warm[:, 0:1], mybir.ActivationFunctionType.Exp, scale=1.0
        )

        # startup DMA critical path
        dma_w_jb(w_k, wk, 0)
        nc.sync.dma_start(b_k[:], bk.rearrange("(jb p) one -> p (jb one)", p=KB))
        ka0, kb0 = k_parts(0, 0)
        ka0()
        kb0()
        dma_w_jb(w_q, wq, 0)
        nc.sync.dma_start(b_q[:], bq.rearrange("(jb p) one -> p (jb one)", p=KB))
        qa0, qb0 = q_parts(0, 0)
        qa0()
        qb0()
        # next DMAs: remaining wk/wq jb slices, then V-path constants
        for jb in range(1, NJB):
            dma_w_jb(w_k, wk, jb)
            dma_w_jb(w_q, wq, jb)
        nc.sync.dma_start(ones8[:], ones_in[:])
        nc.sync.dma_start(ones_bc[:], ones_bc_in[:])
        nc.sync.dma_start(w_v[:], wv.rearrange("p (kb n) -> p kb n", n=HALF))

        # ---- the pipeline ----
        def av_and_drain(d):
            emit_av(d)
            if d.seg_last:
                if d.final:
                    emit_norm(d)
                elif d.tq == 0:
                    emit_spill(d, d.tk < 4)

        n_steps = len(steps)
        av_next = 0
        for s, d in enumerate(steps):
            if d.tq not in ot_gen:
                ot_gen[d.tq] = [
                    p_ot.tile([KB, TB], BF16, tag="ot", name=f"ot{j}")
                    for j in range(NJB)
                ]
            if s == 0:
                emit_sc(d)
            emit_exp(d)
            for thunk in fillers[s]:
                thunk()
            if s + 1 < n_steps:
                emit_sc(steps[s + 1])
            # av stream: lag AV_LAG normally, taper near the end so the
            # post-exp flush (which runs with ACT idle) is short
            want = s - AV_LAG + 1
            if s >= n_steps - 2 * AV_LAG:
                want = s - 1
            while av_next < min(max(want, 0), n_steps) and av_next <= s:
                av_and_drain(steps[av_next])
                av_next += 1
        while av_next < n_steps:
            av_and_drain(steps[av_next])
            av_next += 1
        # tail: out projection of the last t-block, two chains pipelined
        # so DVE copies/DMAs overlap the next pair's matmuls
        def out_pair(tq, specs):
            pos = [p_pj.tile([KB, TB], F32, tag="pj", name="po") for _ in specs]
            for jp in range(NJB):
                for ci, (nb, ts) in enumerate(specs):
                    nc.tensor.matmul(
                        pos[ci][:],
                        ot_gen[tq][jp][:, ts * KB : (ts + 1) * KB],
                        w_o[:, jp, nb * TB : (nb + 1) * TB],
                        start=(jp == 0),
                        stop=(jp == NJB - 1),
                    )
            for ci, (nb, ts) in enumerate(specs):
                st_t = p_st.tile([KB, TB], F32, tag="st", name="st_t")
                nc.vector.tensor_copy(st_t[:], pos[ci][:])
                nc.sync.dma_start(
                    partial[
                        tq * TB + ts * KB : tq * TB + (ts + 1) * KB,
                        nb * TB : (nb + 1) * TB,
                    ],
                    st_t[:],
                )

        tail_specs = [(nb, ts) for nb in range(2) for ts in range(4)]
        for k in range(0, 8, 2):
            out_pair(NTB - 1, tail_specs[k : k + 2])

    nc.compile()
    return nc


def kernel(**inputs: np.ndarray) -> np.ndarray:
    query = np.asarray(inputs["query"], dtype=np.float32)
    key = np.asarray(inputs["key"], dtype=np.float32)
    value = np.asarray(inputs["value"], dtype=np.float32)
    w_q = np.asarray(inputs["w_q"], dtype=np.float32)
    b_q = np.asarray(inputs["b_q"], dtype=np.float32)
    w_k = np.asarray(inputs["w_k"], dtype=np.float32)
    b_k = np.asarray(inputs["b_k"], dtype=np.float32)
    w_v = np.asarray(inputs["w_v"], dtype=np.float32)
    b_v = np.asarray(inputs["b_v"], dtype=np.float32)
    w_o = np.asarray(inputs["w_o"], dtype=np.float32)
    b_o = np.asarray(inputs["b_o"], dtype=np.float32)

    nc = build_kernel()

    bf = ml_dtypes.bfloat16

    def tile_x(a):
        # [T, D] -> transpose -> [kb][tb][128][512] contiguous
        at = a.T.astype(bf)  # [D, T]
        return np.ascontiguousarray(
            at.reshape(NKB, KB, NTB, TB).transpose(2, 1, 0, 3)
        ).reshape(NTB * KB, NKB * TB)

    def tile_w(w_sl):
        # w[sl,:] -> [D, HALF] transposed -> [jb][kb][128][128] contiguous
        wt = w_sl.T.astype(bf)  # [D, HALF]
        return np.ascontiguousarray(
            wt.reshape(NKB, KB, NJB, KB).transpose(2, 1, 0, 3)
        ).reshape(NJB * KB, NKB * KB)

    in_maps = []
    for c in range(N_CORES):
        b = c // 2
        hh = c % 2
        sl = slice(hh * HALF, (hh + 1) * HALF)
        in_maps.append(
            {
                "xq": tile_x(query[b]),
                "xk": tile_x(key[b]),
                "xv": tile_x(value[b]),
                "wq": tile_w(w_q[sl, :]),
                "wk": tile_w(w_k[sl, :]),
                "wv": np.ascontiguousarray(
                    w_v[sl, :].T.astype(bf).reshape(NKB, KB, HALF).transpose(1, 0, 2)
                ).reshape(KB, NKB * HALF),
                "wo": np.ascontiguousarray(
                    w_o[:, sl].T.astype(bf).reshape(NJB, KB, D).transpose(1, 0, 2)
                ).reshape(KB, NJB * D),
                "bq": np.ascontiguousarray(b_q[sl].reshape(HALF, 1)),
                "bk": np.ascontiguousarray(b_k[sl].reshape(HALF, 1)),
                "ones_in": np.ones((KB, H // 2), dtype=bf),
                "ones_bc_in": np.ones((DH + 1, DH), dtype=bf),
            }
        )

    res = run_bass_kernel_spmd(nc, in_maps, core_ids=list(range(N_CORES)))

    const_row = (b_v[None, :] @ w_o.T + b_o[None, :]).astype(np.float32)
    out = np.empty((B, T, D), dtype=np.float32)
    for b in range(B):
        out[b] = res.results[2 * b]["partial"] + res.results[2 * b + 1]["partial"]
        out[b] += const_row
    return out


# revision 17
# speedup vs baseline: 1.0615x; 1.0594x over previous
"""Multi-head attention (B=4, T=2048, D=1024, H=16) on 8 TRN2 NeuronCores.

Sharding: core c handles batch b = c//2 and head-half hh = c%2 (8 heads,
512 of the 1024 channel dims). Each core computes its half of the head
outputs and a row-sharded output projection, producing a partial
[T, D] output. Host unshard: out[b] = partial[2b] + partial[2b+1]
+ b_o + b_v @ w_o.T (the value-bias contribution commutes through
attention because softmax rows sum to 1).

v8: flat software-pipelined emission, ACT-saturating schedule.
  - The ACT exp stream (256 x [128,1024] activations, ~1.3us each) is the
    critical engine; the driver emits one attention step per exp and
    weaves all projection work into PE slack between steps.
  - Startup critical path: only xk/xq of t-block 0 plus the jb0 weight
    slices (~2.5MB) gate the first exp. Host pre-tiles inputs so every
    DMA source is a contiguous block.
  - tq0's attention is chunked by key-t-block (SBUF accumulation) so it
    streams while K/V of t-blocks 1-3 are still being projected.
  - Scores pairs run concurrently in PE row groups 0/64; softmax
    denominator rides row 64 of the AV accumulators (ones column in V);
    a K=1 PE matmul broadcasts it across partitions.
"""

from contextlib import ExitStack

import numpy as np
import ml_dtypes

import concourse.bass as bass
import concourse.mybir as mybir
import concourse.tile as tile
from concourse import bacc
from concourse.bass_utils import run_bass_kernel_spmd

B, T, D = 4, 2048, 1024
H = 16
DH = 64  # head dim
HALF = 512  # channels per core (8 heads)
N_CORES = 8

F32 = mybir.dt.float32
BF16 = mybir.dt.bfloat16

TB = 512  # t-block for moving operands
NTB = T // TB  # 4
KB = 128  # contraction block
NKB = D // KB  # 8
NJB = HALF // KB  # 4 j-blocks of the half
NTK = T // KB  # 16 tk blocks


class Step:
    __slots__ = ("tq", "jp", "tk", "seg_first", "seg_last", "final", "sc", "ex", "seg")

    def __init__(self, tq, jp, tk, seg_first, seg_last, final):
        self.tq = tq
        self.jp = jp
        self.tk = tk
        self.seg_first = seg_first  # first step of an av accumulation segment
        self.seg_last = seg_last  # last step of an av accumulation segment
        self.final = final  # last segment of this (tq, jp): normalize after
        self.sc = None
        self.ex = None
        self.seg = None


def build_kernel():
    nc = bacc.Bacc(
        "TRN2", target_bir_lowering=False, debug=False, num_devices=N_CORES
    )
    # pre-tiled inputs: x*[kb][tb] -> [128, 512] contiguous blocks
    xq = nc.dram_tensor("xq", [NTB * KB, NKB * TB], BF16, kind="ExternalInput").ap()
    xk = nc.dram_tensor("xk", [NTB * KB, NKB * TB], BF16, kind="ExternalInput").ap()
    xv = nc.dram_tensor("xv", [NTB * KB, NKB * TB], BF16, kind="ExternalInput").ap()
    # wq/wk tiled [jb][kb] -> [128, 128] contiguous blocks
    wq = nc.dram_tensor("wq", [NJB * KB, NKB * KB], BF16, kind="ExternalInput").ap()
    wk = nc.dram_tensor("wk", [NJB * KB, NKB * KB], BF16, kind="ExternalInput").ap()
    # wv rows contiguous per kb block; wo rows contiguous per jb block
    wv = nc.dram_tensor("wv", [KB, NKB * HALF], BF16, kind="ExternalInput").ap()
    wo = nc.dram_tensor("wo", [KB, NJB * D], BF16, kind="ExternalInput").ap()
    bq = nc.dram_tensor("bq", [HALF, 1], F32, kind="ExternalInput").ap()
    bk = nc.dram_tensor("bk", [HALF, 1], F32, kind="ExternalInput").ap()
    red_in = nc.dram_tensor("red_in", [KB, 4], BF16, kind="ExternalInput").ap()
    bc2_in = nc.dram_tensor("bc2_in", [2, KB], BF16, kind="ExternalInput").ap()
    partial = nc.dram_tensor("partial", [T, D], F32, kind="ExternalOutput").ap()

    with tile.TileContext(nc) as tc, ExitStack() as ctx:
        p_const = ctx.enter_context(tc.tile_pool(name="const", bufs=1))
        p_kt = ctx.enter_context(tc.tile_pool(name="kt", bufs=NJB * NTB))
        p_v = ctx.enter_context(tc.tile_pool(name="v", bufs=NTK))
        p_qt = ctx.enter_context(tc.tile_pool(name="qt", bufs=2 * NJB))
        p_xs = ctx.enter_context(tc.tile_pool(name="xs", bufs=5))
        p_ex = ctx.enter_context(tc.tile_pool(name="ex", bufs=8))
        p_ot = ctx.enter_context(tc.tile_pool(name="ot", bufs=2 * NJB))
        p_as = ctx.enter_context(tc.tile_pool(name="as", bufs=6))
        p_ac = ctx.enter_context(tc.tile_pool(name="ac", bufs=8))
        p_rc = ctx.enter_context(tc.tile_pool(name="rc", bufs=3))
        p_st = ctx.enter_context(tc.tile_pool(name="st", bufs=2))
        p_da = ctx.enter_context(tc.tile_pool(name="da", bufs=6))
        # PSUM: scores 2x[128,1024] (4 banks) + av 2x[65,512] + pj 2x[128,512]
        p_sc = ctx.enter_context(tc.tile_pool(name="sc", bufs=2, space="PSUM"))
        p_av = ctx.enter_context(tc.tile_pool(name="av", bufs=2, space="PSUM"))
        p_pj = ctx.enter_context(tc.tile_pool(name="pj", bufs=2, space="PSUM"))

        # ---- persistent SBUF tiles ----
        w_k = p_const.tile([KB, NJB, NKB, KB], BF16, tag="wk")
        w_q = p_const.tile([KB, NJB, NKB, KB], BF16, tag="wq")
        w_v = p_const.tile([KB, NKB, HALF], BF16, tag="wv")
        w_o = p_const.tile([KB, NJB, D], BF16, tag="wo")
        b_k = p_const.tile([KB, NJB], F32, tag="bk")
        b_q = p_const.tile([KB, NJB], F32, tag="bq")
        red = p_const.tile([KB, 4], BF16, tag="red")  # [:,0:2]=[1,0] [:,2:4]=[0,1]
        bc2 = p_const.tile([2, KB], BF16, tag="bc2")  # row i: ones on cols 64i..64i+63
        warm = p_const.tile([1, 8], F32, tag="warm")
        warm_o = p_const.tile([1, 8], BF16, tag="warmo")

        kt_tiles = [
            [p_kt.tile([KB, TB], BF16, tag="kt", name=f"kt{j}_{tb}") for tb in range(NTB)]
            for j in range(NJB)
        ]
        v_tiles = [
            p_v.tile([KB, H // 2, DH], BF16, tag="v", name=f"v{j}")
            for j in range(NTK)
        ]

        # ---- DMA emitters ----
        def dma_w_jb(dst, src, jb):
            # one jb slice of wq/wk in a single DMA: [128, kb 8, 128]
            nc.sync.dma_start(
                dst[:, jb],
                src[jb * KB : (jb + 1) * KB, :].rearrange(
                    "p (kb j) -> p kb j", j=KB
                ),
            )

        def load_x_tile(src, tb):
            xt = p_xs.tile([KB, NKB, TB], BF16, tag="xs")
            nc.sync.dma_start(
                xt[:],
                src[tb * KB : (tb + 1) * KB, :].rearrange(
                    "p (kb t) -> p kb t", t=TB
                ),
            )
            return xt

        # ---- projection chain emitters (each returns two half-chain thunks
        # so the driver can spread chains across pipeline steps) ----
        xk_tiles = {}  # tb -> tiles (shared by the 4 jb chains)
        xq_tiles = {}
        xv_tiles = {}
        HC = NKB // 2  # MMs per half-chain

        def k_parts(tb, jb):
            st = {}

            def a():
                if tb not in xk_tiles:
                    xk_tiles[tb] = load_x_tile(xk, tb)
                st["ps"] = p_pj.tile([KB, TB], F32, tag="pj", name="ps")
                for kb in range(HC):
                    nc.tensor.matmul(
                        st["ps"][:],
                        w_k[:, jb, kb, :],
                        xk_tiles[tb][:, kb, :],
                        start=(kb == 0),
                        stop=False,
                    )

            def b():
                for kb in range(HC, NKB):
                    nc.tensor.matmul(
                        st["ps"][:],
                        w_k[:, jb, kb, :],
                        xk_tiles[tb][:, kb, :],
                        start=False,
                        stop=(kb == NKB - 1),
                    )
                nc.vector.tensor_scalar_add(
                    kt_tiles[jb][tb][:], st["ps"][:], b_k[:, jb : jb + 1]
                )
                if jb == NJB - 1:
                    del xk_tiles[tb]

            return a, b

        def v_parts(tb, ts):
            st = {}

            def a():
                if tb not in xv_tiles:
                    xv_tiles[tb] = load_x_tile(xv, tb)
                st["ps"] = p_pj.tile([KB, TB], F32, tag="pj", name="ps")
                for kb in range(HC):
                    nc.tensor.matmul(
                        st["ps"][:],
                        xv_tiles[tb][:, kb, ts * KB : (ts + 1) * KB],
                        w_v[:, kb, :],
                        start=(kb == 0),
                        stop=False,
                    )

            def b():
                for kb in range(HC, NKB):
                    nc.tensor.matmul(
                        st["ps"][:],
                        xv_tiles[tb][:, kb, ts * KB : (ts + 1) * KB],
                        w_v[:, kb, :],
                        start=False,
                        stop=(kb == NKB - 1),
                    )
                nc.vector.tensor_copy(
                    v_tiles[tb * 4 + ts][:],
                    st["ps"][:].rearrange("p (h d) -> p h d", d=DH),
                )
                if ts == 3:
                    del xv_tiles[tb]

            return a, b

        qt_gen = {}  # tq -> list of qt tiles

        def q_parts(tq, jb):
            st = {}

            def a():
                if tq not in qt_gen:
                    qt_gen[tq] = [None] * NJB
                if tq not in xq_tiles:
                    xq_tiles[tq] = load_x_tile(xq, tq)
                st["ps"] = p_pj.tile([KB, TB], F32, tag="pj", name="ps")
                for kb in range(HC):
                    nc.tensor.matmul(
                        st["ps"][:],
                        w_q[:, jb, kb, :],
                        xq_tiles[tq][:, kb, :],
                        start=(kb == 0),
                        stop=False,
                    )

            def b():
                for kb in range(HC, NKB):
                    nc.tensor.matmul(
                        st["ps"][:],
                        w_q[:, jb, kb, :],
                        xq_tiles[tq][:, kb, :],
                        start=False,
                        stop=(kb == NKB - 1),
                    )
                qt = p_qt.tile([KB, TB], BF16, tag="qt", name=f"qt{jb}")
                nc.vector.tensor_scalar_add(qt[:], st["ps"][:], b_q[:, jb : jb + 1])
                qt_gen[tq][jb] = qt
                if jb == NJB - 1:
                    del xq_tiles[tq]

            return a, b

        # ---- attention step emitters ----
        def emit_sc(d):
            d.sc = p_sc.tile([KB, 2 * TB], F32, tag="sc", name="sc")
            qt = qt_gen[d.tq]
            for i in range(2):
                nc.tensor.matmul(
                    d.sc[:, i * TB : (i + 1) * TB],
                    kt_tiles[d.jp][d.tk // 4][
                        i * DH : (i + 1) * DH,
                        (d.tk % 4) * KB : (d.tk % 4 + 1) * KB,
                    ],
                    qt[d.jp][i * DH : (i + 1) * DH, :],
                    start=True,
                    stop=True,
                )

        def emit_exp(d):
            d.ex = p_ex.tile([KB, 2 * TB], BF16, tag="ex", name="ex")
            nc.scalar.activation(
                d.ex[:], d.sc[:], mybir.ActivationFunctionType.Exp, scale=0.125
            )

        seg_avs = {}  # (tq, jp) -> current av psum tile [128, 512]
        daccs = {}  # (tq, jp) -> bf16 running sum of ex tiles

        def emit_av(d):
            if d.seg_first:
                seg_avs[d.tq, d.jp] = p_av.tile(
                    [KB, TB], F32, tag="av", name="avp"
                )
            avp = seg_avs[d.tq, d.jp]
            for i in range(2):
                nc.tensor.matmul(
                    avp[i * DH : (i + 1) * DH, :],
                    v_tiles[d.tk][:, 2 * d.jp + i, :],
                    d.ex[:, i * TB : (i + 1) * TB],
                    start=d.seg_first,
                    stop=d.seg_last,
                )
            # denominator partials: elementwise bf16 accumulation of ex
            first = d.seg_first and (d.tq != 0 or d.tk < 4)
            if first:
                daccs[d.tq, d.jp] = p_da.tile(
                    [KB, 2 * TB], BF16, tag="da", name="dacc"
                )
                nc.vector.tensor_copy(daccs[d.tq, d.jp][:], d.ex[:])
            else:
                nc.vector.tensor_add(
                    daccs[d.tq, d.jp][:], daccs[d.tq, d.jp][:], d.ex[:]
                )

        acc = {}  # (tq, jp) -> sbuf accumulator (chunked tq only)
        ot_gen = {}  # tq -> ot tiles

        def emit_spill(d, first_chunk):
            """Chunked path: move/add av psum into the SBUF accumulator."""
            avp = seg_avs.pop((d.tq, d.jp))
            if first_chunk:
                acc[d.tq, d.jp] = p_ac.tile(
                    [KB, TB], F32, tag="ac", name=f"ac{d.jp}"
                )
                nc.vector.tensor_copy(acc[d.tq, d.jp][:], avp[:])
            else:
                nc.vector.tensor_add(acc[d.tq, d.jp][:], acc[d.tq, d.jp][:], avp[:])
            return avp

        def emit_norm(d):
            """Final segment of (tq, jp): drain av, reduce+broadcast the
            denominator inside the same psum tile, normalize into ot."""
            if d.tq == 0:
                avp = emit_spill(d, False)
                src = acc.pop((d.tq, d.jp))
            else:
                avp = seg_avs.pop((d.tq, d.jp))
                src = p_as.tile([KB, TB], F32, tag="as", name="spl")
                nc.vector.tensor_copy(src[:], avp[:])
            dacc = daccs.pop((d.tq, d.jp))
            # denom[i] = sum over partitions of dacc head column block i
            nc.tensor.matmul(
                avp[0:2, :], red[:, 0:2], dacc[:, 0:TB], start=True, stop=False
            )
            nc.tensor.matmul(
                avp[0:2, :], red[:, 2:4], dacc[:, TB : 2 * TB], start=False, stop=True
            )
            rc2 = p_rc.tile([2, TB], F32, tag="rc2")
            nc.vector.reciprocal_approx_fast(rc2[:], avp[0:2, :])
            rcb = p_rc.tile([2, TB], BF16, tag="rcb")
            nc.vector.tensor_copy(rcb[:], rc2[:])
            nc.tensor.matmul(avp[:], bc2[:], rcb[:], start=True, stop=True)
            nc.vector.tensor_mul(ot_gen[d.tq][d.jp][:], src[:], avp[:])

        def out_parts(tq, nb, ts):
            stt = {}

            def mm_half(lo, hi):
                for jp in range(lo, hi):
                    nc.tensor.matmul(
                        stt["po"][:],
                        ot_gen[tq][jp][:, ts * KB : (ts + 1) * KB],
                        w_o[:, jp, nb * TB : (nb + 1) * TB],
                        start=(jp == 0),
                        stop=(jp == NJB - 1),
                    )

            def a():
                stt["po"] = p_pj.tile([KB, TB], F32, tag="pj", name="po")
                mm_half(0, 2)

            def b():
                mm_half(2, NJB)
                st = p_st.tile([KB, TB], F32, tag="st")
                nc.vector.tensor_copy(st[:], stt["po"][:])
                nc.sync.dma_start(
                    partial[
                        tq * TB + ts * KB : tq * TB + (ts + 1) * KB,
                        nb * TB : (nb + 1) * TB,
                    ],
                    st[:],
                )

            return a, b

        # ---- build the global step list ----
        steps = []
        for tq in range(NTB):
            if tq == 0:
                for ci in range(4):
                    for jp in range(NJB):
                        for u in range(4):
                            steps.append(
                                Step(tq, jp, ci * 4 + u, u == 0, u == 3, ci == 3)
                            )
            else:
                for jp in range(NJB):
                    for u in range(NTK):
                        steps.append(Step(tq, jp, u, u == 0, u == NTK - 1, True))

        # ---- per-step filler plan: fillers[s] emitted between exp(s) and
        # sc(s+1).  Deadlines: a chain feeding sc(s+1) must be in fillers[<=s];
        # one feeding av(s-LAG) must be in fillers[<=s] as well (av comes last).
        AV_LAG = 6
        fillers = [[] for _ in steps]

        def place(s, thunk):
            fillers[min(max(s, 0), len(steps) - 1)].append(thunk)

        # tq0: K/Q chains jb1-3 (sc(4*jb) lookahead at step 4*jb-1)
        for j, s0 in [(1, 0), (2, 4), (3, 8)]:
            ka, kb_ = k_parts(0, j)
            qa, qb = q_parts(0, j)
            place(s0, ka)
            place(s0 + 1, kb_)
            place(s0 + 2, qa)
            place(s0 + 3, qb)
        # V tb0: av(u) runs at step u+AV_LAG
        for u in range(4):
            va, vb = v_parts(0, u)
            place(2 + 2 * u, va)
            place(3 + 2 * u, vb)
        def prefetch(xt_dict, dram, tb):
            def emit():
                if tb not in xt_dict:
                    xt_dict[tb] = load_x_tile(dram, tb)
            return emit

        # K/V of tb 1-3: kt[jp][tb] first read by sc at step 16*tb+4*jp-1;
        # v_tiles[4*tb+u] first read by av at step 16*tb+u+AV_LAG.
        for tb in range(1, NTB):
            place(16 * tb - 12, prefetch(xk_tiles, xk, tb))
            place(16 * tb - 4, prefetch(xv_tiles, xv, tb))
            for j in range(NJB):
                ka, kb_ = k_parts(tb, j)
                place(16 * tb - 6 + 2 * j, ka)
                place(16 * tb - 5 + 2 * j, kb_)
            for u in range(4):
                va, vb = v_parts(tb, u)
                place(16 * tb + 2 + 2 * u, va)
                place(16 * tb + 3 + 2 * u, vb)
        place(30, lambda: nc.sync.dma_start(
            w_o[:], wo.rearrange("p (jb n) -> p jb n", n=D)))
        # q_proj(tq+1) near the end of tq; out_proj(tq-1) spread across tq.
        for tq in range(NTB):
            base = tq * 64
            if tq + 1 < NTB:
                place(base + 44, prefetch(xq_tiles, xq, tq + 1))
                for jb in range(NJB):
                    qa, qb = q_parts(tq + 1, jb)
                    place(base + 50 + 3 * jb, qa)
                    place(base + 51 + 3 * jb, qb)
            if tq > 0:
                for idx, (nb, ts) in enumerate(
                    (nb, ts) for nb in range(2) for ts in range(4)
                ):
                    oa, ob = out_parts(tq - 1, nb, ts)
                    place(base + 5 + 4 * idx, oa)
                    place(base + 6 + 4 * idx, ob)

        # ---- emission ----
        # ACT table pre-warm: a tiny exp long before the first real one
        nc.sync.dma_start(warm[:, 0:1], bq[0:1, 0:1])
        nc.scalar.activation(
            warm_o[:, 0:1], warm[:, 0:1], mybir.ActivationFunctionType.Exp, scale=1.0
        )

        # startup DMA critical path
        dma_w_jb(w_k, wk, 0)
        nc.sync.dma_start(b_k[:], bk.rearrange("(jb p) one -> p (jb one)", p=KB))
        ka0, kb0 = k_parts(0, 0)
        ka0()
        kb0()
        dma_w_jb(w_q, wq, 0)
        nc.sync.dma_start(b_q[:], bq.rearrange("(jb p) one -> p (jb one)", p=KB))
        qa0, qb0 = q_parts(0, 0)
        qa0()
        qb0()
        # next DMAs: remaining wk/wq jb slices, then V-path constants
        for jb in range(1, NJB):
            dma_w_jb(w_k, wk, jb)
            dma_w_jb(w_q, wq, jb)
        nc.sync.dma_start(red[:], red_in[:])
        nc.sync.dma_start(bc2[:], bc2_in[:])
        nc.sync.dma_start(w_v[:], wv.rearrange("p (kb n) -> p kb n", n=HALF))

        # ---- the pipeline ----
        def av_and_drain(d):
            emit_av(d)
            if d.seg_last:
                if d.final:
                    emit_norm(d)
                elif d.tq == 0:
                    emit_spill(d, d.tk < 4)

        n_steps = len(steps)
        av_next = 0
        for s, d in enumerate(steps):
            if d.tq not in ot_gen:
                ot_gen[d.tq] = [
                    p_ot.tile([KB, TB], BF16, tag="ot", name=f"ot{j}")
                    for j in range(NJB)
                ]
            if s == 0:
                emit_sc(d)
            emit_exp(d)
            for thunk in fillers[s]:
                thunk()
            if s + 1 < n_steps:
                emit_sc(steps[s + 1])
            # av stream: lag AV_LAG normally, taper near the end so the
            # post-exp flush (which runs with ACT idle) is short
            want = s - AV_LAG + 1
            if s >= n_steps - 2 * AV_LAG:
                want = s - 1
            while av_next < min(max(want, 0), n_steps) and av_next <= s:
                av_and_drain(steps[av_next])
                av_next += 1
        while av_next < n_steps:
            av_and_drain(steps[av_next])
            av_next += 1
        # tail: out projection of the last t-block, two chains pipelined
        # so DVE copies/DMAs overlap the next pair's matmuls
        def out_pair(tq, specs):
            pos = [p_pj.tile([KB, TB], F32, tag="pj", name="po") for _ in specs]
            for jp in range(NJB):
                for ci, (nb, ts) in enumerate(specs):
                    nc.tensor.matmul(
                        pos[ci][:],
                        ot_gen[tq][jp][:, ts * KB : (ts + 1) * KB],
                        w_o[:, jp, nb * TB : (nb + 1) * TB],
                        start=(jp == 0),
                        stop=(jp == NJB - 1),
                    )
            for ci, (nb, ts) in enumerate(specs):
                st_t = p_st.tile([KB, TB], F32, tag="st", name="st_t")
                nc.vector.tensor_copy(st_t[:], pos[ci][:])
                nc.sync.dma_start(
                    partial[
                        tq * TB + ts * KB : tq * TB + (ts + 1) * KB,
                        nb * TB : (nb + 1) * TB,
                    ],
                    st_t[:],
                )

        tail_specs = [(nb, ts) for nb in range(2) for ts in range(4)]
        for k in range(0, 8, 2):
            out_pair(NTB - 1, tail_specs[k : k + 2])

    nc.compile()
    return nc


def kernel(**inputs: np.ndarray) -> np.ndarray:
    query = np.asarray(inputs["query"], dtype=np.float32)
    key = np.asarray(inputs["key"], dtype=np.float32)
    value = np.asarray(inputs["value"], dtype=np.float32)
    w_q = np.asarray(inputs["w_q"], dtype=np.float32)
    b_q = np.asarray(inputs["b_q"], dtype=np.float32)
    w_k = np.asarray(inputs["w_k"], dtype=np.float32)
    b_k = np.asarray(inputs["b_k"], dtype=np.float32)
    w_v = np.asarray(inputs["w_v"], dtype=np.float32)
    b_v = np.asarray(inputs["b_v"], dtype=np.float32)
    w_o = np.asarray(inputs["w_o"], dtype=np.float32)
    b_o = np.asarray(inputs["b_o"], dtype=np.float32)

    nc = build_kernel()

    bf = ml_dtypes.bfloat16

    def tile_x(a):
        # [T, D] -> transpose -> [kb][tb][128][512] contiguous
        at = a.T.astype(bf)  # [D, T]
        return np.ascontiguousarray(
            at.reshape(NKB, KB, NTB, TB).transpose(2, 1, 0, 3)
        ).reshape(NTB * KB, NKB * TB)

    def tile_w(w_sl):
        # w[sl,:] -> [D, HALF] transposed -> [jb][kb][128][128] contiguous
        wt = w_sl.T.astype(bf)  # [D, HALF]
        return np.ascontiguousarray(
            wt.reshape(NKB, KB, NJB, KB).transpose(2, 1, 0, 3)
        ).reshape(NJB * KB, NKB * KB)

    red_host = np.zeros((KB, 4), dtype=bf)
    red_host[:, 0] = 1
    red_host[:, 3] = 1
    bc2_host = np.zeros((2, KB), dtype=bf)
    bc2_host[0, 0:DH] = 1
    bc2_host[1, DH:KB] = 1

    in_maps = []
    for c in range(N_CORES):
        b = c // 2
        hh = c % 2
        sl = slice(hh * HALF, (hh + 1) * HALF)
        in_maps.append(
            {
                "xq": tile_x(query[b]),
                "xk": tile_x(key[b]),
                "xv": tile_x(value[b]),
                "wq": tile_w(w_q[sl, :]),
                "wk": tile_w(w_k[sl, :]),
                "wv": np.ascontiguousarray(
                    w_v[sl, :].T.astype(bf).reshape(NKB, KB, HALF).transpose(1, 0, 2)
                ).reshape(KB, NKB * HALF),
                "wo": np.ascontiguousarray(
                    w_o[:, sl].T.astype(bf).reshape(NJB, KB, D).transpose(1, 0, 2)
                ).reshape(KB, NJB * D),
                "bq": np.ascontiguousarray(b_q[sl].reshape(HALF, 1)),
                "bk": np.ascontiguousarray(b_k[sl].reshape(HALF, 1)),
                "red_in": red_host,
                "bc2_in": bc2_host,
            }
        )

    res = run_bass_kernel_spmd(nc, in_maps, core_ids=list(range(N_CORES)))

    const_row = (b_v[None, :] @ w_o.T + b_o[None, :]).astype(np.float32)
    out = np.empty((B, T, D), dtype=np.float32)
    for b in range(B):
        out[b] = res.results[2 * b]["partial"] + res.results[2 * b + 1]["partial"]
        out[b] += const_row
    return out
